# revision 1
# baseline (speedup 1.0000x reference)
"""Trainium2 kernel: X = inv(phi + sigma2*A) for the DeepKernelPacketGP module.

Host (f64, O(n) prep): pentadiagonal bands of B via batched 5x5 kernel-packet
window solves; boundary Riccati scans; dressed leaf inverses; per-tree-node
2x2 chain coefficients + dressed interface strips.
Device (fp32, O(n^2) work, 8 cores, column-slab sharding): log-depth boundary
-row chains down a bisection tree + all leaf row-block writes via PE matmuls;
each core materializes X[:, core*512:(core+1)*512].
"""
import sys
sys.path.insert(0, '/opt/trn_rl_repo')
import numpy as np

N = 4096
NB = 64                    # leaf span size
NLEAF = N // NB            # 64
LEVELS = 6                 # 2^6 leaves
NCORES = 8
SLAB = N // NCORES         # 512

# ============================================================================
# Host math (float64)
# ============================================================================

def _stage1_bands(x, rho, sigma2):
    n = x.shape[0]; k = 5; m = 2; n_pow = 2
    c = np.sqrt(3.0) / rho
    W = n - 4
    idx = np.arange(W)[:, None] + np.arange(k)[None, :]
    xw = x[idx]
    t = xw - (xw[:, :1] + xw[:, -1:]) / 2
    pw = t[:, :, None] ** np.arange(n_pow)
    pos = pw * np.exp(c * t)[:, :, None]
    neg = pw * np.exp(-c * t)[:, :, None]
    e_first = np.zeros((W, 1, k)); e_first[:, :, 0] = 1.0
    Amat = np.concatenate([np.swapaxes(pos, 1, 2), np.swapaxes(neg, 1, 2), e_first], axis=1)
    rhs = np.zeros((k,)); rhs[-1] = 1.0
    a = np.linalg.solve(Amat, np.broadcast_to(rhs, (W, k))[..., None])[..., 0]
    d = np.abs(xw[:, :, None] - xw[:, None, :]); s = c * d
    Kw = (1 + s) * np.exp(-s)
    phiv = np.einsum('wij,wj->wi', Kw, a)
    bcol = phiv + sigma2 * a
    Bcols = np.zeros((n, 5))
    Bcols[2:n-2, :] = bcol
    def bnd(xseg, tshift, npos, nneg):
        ss = xseg.shape[0]
        xt = xseg + tshift
        rows = [xt**j * np.exp(c*xt) for j in range(npos)]
        rows += [xt**j * np.exp(-c*xt) for j in range(nneg)]
        e = np.zeros(ss); e[0] = 1.0
        rows.append(e)
        M = np.stack(rows); r = np.zeros(ss); r[-1] = 1.0
        aa = np.linalg.solve(M, r)
        dd = np.abs(xseg[:, None] - xseg[None, :]); s2 = c*dd
        return aa, ((1+s2)*np.exp(-s2)) @ aa
    for i in range(m):
        s_l = i + m + 1
        aa, pp = bnd(x[:s_l], -x[s_l-1], n_pow, s_l - 3)
        for r in range(s_l):
            Bcols[i, r - i + 2] = pp[r] + sigma2*aa[r]
        s_r = k - 1 - i
        aa, pp = bnd(x[n-s_r:], -x[n-s_r], s_r - 3, n_pow)
        col = n - m + i
        for ridx in range(s_r):
            r = n - s_r + ridx
            Bcols[col, r - col + 2] = pp[ridx] + sigma2*aa[ridx]
    return Bcols


def _bands_by_diag(Bcols):
    n = Bcols.shape[0]
    bd = {d: np.zeros(n) for d in range(-2, 3)}
    for j in range(5):
        c0 = max(0, 2 - j); c1 = min(n, n + 2 - j)
        for col in range(c0, c1):
            r = col - 2 + j
            bd[col - r][r] = Bcols[col, j]
    return bd


def _span_matrix(bd, lo, hi):
    s = hi - lo
    M = np.zeros((s, s))
    for d in range(-2, 3):
        r0 = max(0, -d); r1 = min(s, s - d)
        rr = np.arange(r0, r1)
        M[rr, rr + d] = bd[d][lo + rr]
    return M


def _c_coup(bd, b):
    return np.array([[bd[2][b-2], 0.0], [bd[1][b-1], bd[2][b-1]]])


def _e_coup(bd, b):
    return np.array([[bd[-2][b], bd[-1][b]], [0.0, bd[-2][b+1]]])


def _banded_solve(bd, lo, hi, tl, br, rhs, transpose=False):
    """Solve (B_span - mods) X = rhs (dense np solve for simplicity on small
    spans; LU via scipy-free banded elimination for big spans)."""
    D = _span_matrix(bd, lo, hi)
    if tl is not None: D[:2, :2] -= tl
    if br is not None: D[-2:, -2:] -= br
    if transpose:
        D = D.T
    return np.linalg.solve(D, rhs)


def _host_pieces(bd):
    n = N; nl = NLEAF
    GL = np.zeros((nl+1, 2, 2))
    for k in range(1, nl+1):
        lo = (k-1)*NB
        D = _span_matrix(bd, lo, lo+NB)
        if k > 1:
            D[:2, :2] -= _e_coup(bd, lo) @ GL[k-1] @ _c_coup(bd, lo)
        GL[k] = np.linalg.inv(D)[-2:, -2:]
    GR = np.zeros((nl+1, 2, 2))
    for k in range(nl-1, -1, -1):
        lo = k*NB
        D = _span_matrix(bd, lo, lo+NB)
        if k < nl-1:
            b = lo + NB
            D[-2:, -2:] -= _c_coup(bd, b) @ GR[k+1] @ _e_coup(bd, b)
        GR[k] = np.linalg.inv(D)[:2, :2]
    Xhat = np.zeros((nl, NB, NB))
    gTLe = np.zeros((nl, NB, 2))
    gBRe = np.zeros((nl, NB, 2))
    for ell in range(nl):
        lo = ell*NB; hi = lo + NB
        D0 = _span_matrix(bd, lo, hi)
        TLm = np.zeros((NB, NB)); BRm = np.zeros((NB, NB))
        if lo > 0:
            TLm[:2, :2] = _e_coup(bd, lo) @ GL[ell] @ _c_coup(bd, lo)
        if hi < n:
            BRm[-2:, -2:] = _c_coup(bd, hi) @ GR[ell+1] @ _e_coup(bd, hi)
        Xhat[ell] = np.linalg.inv(D0 - TLm - BRm)
        if hi < n:
            gTLe[ell] = -np.linalg.inv(D0 - TLm)[:, -2:] @ _c_coup(bd, hi)
        if lo > 0:
            gBRe[ell] = -np.linalg.inv(D0 - BRm)[:, :2] @ _e_coup(bd, lo)

    def dressed_rows(lo, hi, tl, br, rows):
        s = hi - lo
        rhs = np.zeros((s, len(rows)))
        for i, r in enumerate(rows):
            rhs[r, i] = 1.0
        return _banded_solve(bd, lo, hi, tl, br, rhs, transpose=True).T

    nodes = []
    for L in range(1, LEVELS+1):
        sz = NB * 2**L
        cnt = n // sz
        CA = np.zeros((cnt, 2, 2)); DA = np.zeros((cnt, 2, 2))
        CB = np.zeros((cnt, 2, 2)); DB = np.zeros((cnt, 2, 2))
        sA = np.zeros((cnt, 2, sz//2)); sB = np.zeros((cnt, 2, sz//2))
        for i in range(cnt):
            mlo = i*sz; mhi = mlo + sz; mid = mlo + sz//2
            kA0 = mlo//NB; kA1 = mid//NB; kB1 = mhi//NB
            tlA = brB = None
            if mlo > 0:
                tlA = _e_coup(bd, mlo) @ GL[kA0] @ _c_coup(bd, mlo)
            if mhi < n:
                brB = _c_coup(bd, mhi) @ GR[kB1] @ _e_coup(bd, mhi)
            cM = _c_coup(bd, mid); eM = _e_coup(bd, mid)
            brA = cM @ GR[kA1] @ eM
            tlB = eM @ GL[kA1] @ cM
            half = sz//2
            rA = dressed_rows(mlo, mid, tlA, None, [half-2, half-1])
            CA[i] = -(rA[:, -2:]) @ cM
            rB = dressed_rows(mid, mhi, tlB, None, [0, 1])
            CB[i] = -(rB[:, -2:]) @ (_c_coup(bd, mhi) if mhi < n else np.zeros((2, 2)))
            rA2 = dressed_rows(mlo, mid, None, brA, [half-2, half-1])
            DA[i] = -(rA2[:, :2]) @ (_e_coup(bd, mlo) if mlo > 0 else np.zeros((2, 2)))
            rB2 = dressed_rows(mid, mhi, None, brB, [0, 1])
            DB[i] = -(rB2[:, :2]) @ eM
            sA[i] = dressed_rows(mlo, mid, tlA, brA, [half-2, half-1])
            sB[i] = dressed_rows(mid, mhi, tlB, brB, [0, 1])
        nodes.append(dict(CA=CA, DA=DA, CB=CB, DB=DB, sA=sA, sB=sB, sz=sz))
    return dict(GL=GL, GR=GR, Xhat=Xhat, gTLe=gTLe, gBRe=gBRe, nodes=nodes)


def _core_inputs(P, core):
    col_lo = core * SLAB
    cols = np.arange(col_lo, col_lo + SLAB)
    f32 = np.float32
    ins = {}
    for L in range(LEVELS, 0, -1):
        nd = P["nodes"][L-1]
        sz = nd["sz"]; cnt = N // sz
        coef = np.zeros((cnt, 16), f32)
        coef[:, 0:4] = nd["CA"].reshape(cnt, 4)
        coef[:, 4:8] = nd["DA"].reshape(cnt, 4)
        coef[:, 8:12] = nd["CB"].reshape(cnt, 4)
        coef[:, 12:16] = nd["DB"].reshape(cnt, 4)
        ins[f"coef{L}"] = coef
        strip = np.zeros((cnt, 4, SLAB), f32)
        thr = np.zeros((cnt, 4), f32)
        for i in range(cnt):
            mlo = i*sz; mid = mlo + sz//2; mhi = mlo + sz
            thr[i] = [mlo, mid, mhi, 0]
            mA = (cols >= mlo) & (cols < mid)
            mB = (cols >= mid) & (cols < mhi)
            if mA.any():
                strip[i, 0:2, mA] = nd["sA"][i][:, cols[mA]-mlo].astype(f32).T
            if mB.any():
                strip[i, 2:4, mB] = nd["sB"][i][:, cols[mB]-mid].astype(f32).T
        ins[f"strip{L}"] = strip.reshape(cnt, 4*SLAB)
        ins[f"thr{L}"] = thr
    # leaf-write matrices: groups of 2 leaves (128 rows); lhsT [8, 128]
    lmatT = np.zeros((32, 8, 128), f32)
    rmask = np.zeros((32, 8, SLAB), f32)
    for g in range(32):
        for li in range(2):
            ell = 2*g + li
            blk = np.zeros((NB, 4))
            blk[:, 0:2] = P["gTLe"][ell]     # multiplies bl rows
            blk[:, 2:4] = P["gBRe"][ell]     # multiplies ab rows
            lmatT[g, li*4:(li+1)*4, li*NB:(li+1)*NB] = blk.T
            lo = ell*NB; hi = lo + NB
            rmask[g, li*4+0:li*4+2, :] = (cols >= hi).astype(f32)[None, :]
            rmask[g, li*4+2:li*4+4, :] = (cols < lo).astype(f32)[None, :]
    ins["lmatT"] = lmatT
    ins["rmask"] = rmask
    # diag inserts: 4 groups per core; [4, 128, 128]
    xh = np.zeros((4, 128, 128), f32)
    for j in range(4):
        g = core*4 + j
        for li in range(2):
            ell = 2*g + li
            xh[j, li*NB:(li+1)*NB, li*NB:(li+1)*NB] = P["Xhat"][ell]
    ins["xhat"] = xh.transpose(1, 0, 2).reshape(128, 4*128).copy()
    ins["colidx"] = np.broadcast_to(cols.astype(f32), (128, SLAB)).copy()
    dfl = np.zeros((128, 32), f32)
    for j in range(4):
        dfl[:, core*4 + j] = 1.0
    ins["diagflag"] = dfl
    return ins


# ============================================================================
# Device kernel
# ============================================================================

_CACHED = {}

def _build_nc():
    import concourse.bass as bass
    import concourse.mybir as mybir
    import concourse.tile as tile
    from concourse.vector_clock import ScopedClock

    def _patched_drain_and_barrier(self, tick_clock, wait_clock):
        nopw = self.nc.gpsimd.nop()
        wait_clock.add_sem_waits(nopw.ins, ScopedClock({None: tick_clock.global_clock}))
        waits = list(nopw.ins.sync_info.on_wait) if nopw.ins.sync_info else []
        if len(waits) > 1:
            nopw.ins.sync_info.on_wait = waits[:1]
            for w in waits[1:]:
                extra = self.nc.gpsimd.nop()
                extra.ins.sync_info = mybir.SyncInfo(on_wait=[w], on_update=[])
        self.nc.sync.drain()
        self.nc.all_engine_barrier()
        assert self.sems is not None
        popped = self.nc._tile_sem_poison_stack.pop()
        assert popped is self._sem_poison
        self.nc.clear_and_free_semaphores(list(self.sems.allocated().values()))
        self.nc.all_engine_barrier()
    tile.TileContext._drain_and_barrier = _patched_drain_and_barrier

    F32 = mybir.dt.float32
    MUL = mybir.AluOpType.mult
    ADD = mybir.AluOpType.add
    GE = mybir.AluOpType.is_ge
    LT = mybir.AluOpType.is_lt
    S = SLAB

    nc = bass.Bass(target_bir_lowering=False)
    dins = {}
    for L in range(LEVELS, 0, -1):
        cnt = N // (NB * 2**L)
        dins[f"coef{L}"] = nc.dram_tensor(f"coef{L}", [cnt, 16], F32, kind="ExternalInput")
        dins[f"strip{L}"] = nc.dram_tensor(f"strip{L}", [cnt, 4*S], F32, kind="ExternalInput")
        dins[f"thr{L}"] = nc.dram_tensor(f"thr{L}", [cnt, 4], F32, kind="ExternalInput")
    dins["lmatT"] = nc.dram_tensor("lmatT", [32, 8, 128], F32, kind="ExternalInput")
    dins["rmask"] = nc.dram_tensor("rmask", [32, 8, S], F32, kind="ExternalInput")
    dins["xhat"] = nc.dram_tensor("xhat", [128, 4*128], F32, kind="ExternalInput")
    dins["colidx"] = nc.dram_tensor("colidx", [128, S], F32, kind="ExternalInput")
    dins["diagflag"] = nc.dram_tensor("diagflag", [128, 32], F32, kind="ExternalInput")
    dout = nc.dram_tensor("xslab", [N, S], F32, kind="ExternalOutput")

    with tile.TileContext(nc) as tc:
        with tc.tile_pool(name="main", bufs=1) as pool, \
             tc.tile_pool(name="io", bufs=2) as iopool, \
             tc.tile_pool(name="ps", bufs=4, space="PSUM") as pspool:
            colidx = pool.tile([128, S], F32, tag="colidx")
            nc.sync.dma_start(colidx[:], dins["colidx"][:])
            # boundary tiles per span-level: bnd_k has (64/2^k spans)+1 rows
            bnd = {}
            for Lspan in range(LEVELS + 1):
                rows = (N // (NB * 2**Lspan)) + 1
                t = pool.tile([rows, 4*S], F32, tag=f"bnd{Lspan}")
                nc.vector.memset(t[:], 0.0)
                bnd[Lspan] = t
            for L in range(LEVELS, 0, -1):
                cnt = N // (NB * 2**L)
                coef = pool.tile([cnt, 16], F32, tag="coef")
                strip = pool.tile([cnt, 4*S], F32, tag="strip")
                thr = pool.tile([cnt, 4], F32, tag="thr")
                nc.sync.dma_start(coef[:], dins[f"coef{L}"][:])
                nc.sync.dma_start(strip[:], dins[f"strip{L}"][:])
                nc.sync.dma_start(thr[:], dins[f"thr{L}"][:])
                prev = bnd[L]           # [cnt+1, 4S] boundaries of level-L spans
                newb = pool.tile([cnt, 4*S], F32, tag="newb")
                tmp = pool.tile([cnt, 2*S], F32, tag="tmpc")
                tmp2 = pool.tile([cnt, 2*S], F32, tag="tmp2c")
                msk = pool.tile([cnt, S], F32, tag="mskc")
                a2 = prev[0:cnt, 0:2*S]          # u-part of left boundary
                b2 = pool.tile([cnt, 2*S], F32, tag="b2t")
                nc.sync.dma_start(b2[:], prev[1:cnt+1, 2*S:4*S])
                b2 = b2[:]
                u = newb[:, 0:2*S]; v = newb[:, 2*S:4*S]

                def mat2_apply(dst, cbase, src):
                    # dst[:,r*S:(r+1)*S] = c[2r]*src_row0 + c[2r+1]*src_row1
                    for r in range(2):
                        nc.vector.tensor_scalar(
                            tmp2[:, r*S:(r+1)*S], src[:, 0:S],
                            coef[:, cbase+2*r:cbase+2*r+1], None, MUL)
                        nc.vector.tensor_scalar(
                            dst[:, r*S:(r+1)*S], src[:, S:2*S],
                            coef[:, cbase+2*r+1:cbase+2*r+2], None, MUL)
                        nc.vector.tensor_tensor(
                            dst[:, r*S:(r+1)*S], dst[:, r*S:(r+1)*S],
                            tmp2[:, r*S:(r+1)*S], ADD)

                def apply_mask(dst, thr_col, op):
                    nc.vector.tensor_scalar(msk[:], colidx[0:cnt, :],
                                            thr[:, thr_col:thr_col+1], None, op)
                    for r in range(2):
                        nc.vector.tensor_tensor(dst[:, r*S:(r+1)*S],
                                                dst[:, r*S:(r+1)*S], msk[:], MUL)

                # u_a = (DA @ a2)*[col < mlo] + stripA
                mat2_apply(u, 4, a2)
                apply_mask(u, 0, LT)
                nc.vector.tensor_tensor(u, u, strip[:, 0:2*S], ADD)
                # v = (DB @ u_a)*[col < mid] + stripB + (CB @ b2)*[col >= mhi]
                mat2_apply(v, 12, u)
                apply_mask(v, 1, LT)
                nc.vector.tensor_tensor(v, v, strip[:, 2*S:4*S], ADD)
                mat2_apply(tmp, 8, b2)
                apply_mask(tmp, 2, GE)
                nc.vector.tensor_tensor(v, v, tmp[:], ADD)
                # u += (CA @ v)*[col >= mid]
                mat2_apply(tmp, 0, v)
                apply_mask(tmp, 1, GE)
                nc.vector.tensor_tensor(u, u, tmp[:], ADD)
                # interleave into bnd[L-1]: even <- prev, odd <- newb
                nxt = bnd[L-1]
                import concourse.bass as _b
                nc.sync.dma_start(
                    _b.AP(nxt.tensor, nxt.offset, [[2*(4*S), cnt+1], [1, 4*S]]),
                    prev[0:cnt+1, :])
                nc.sync.dma_start(
                    _b.AP(nxt.tensor, nxt.offset + 4*S, [[2*(4*S), cnt], [1, 4*S]]),
                    newb[:, :])
            bleaf = bnd[0]   # [65, 4S]
            # ---- leaf writes ----
            import concourse.bass as _b
            xh = pool.tile([128, 4*128], F32, tag="xh")
            nc.sync.dma_start(xh[:], dins["xhat"][:])
            dfl = pool.tile([128, 32], F32, tag="dfl")
            nc.sync.dma_start(dfl[:], dins["diagflag"][:])
            # R-all [8, 32*S]: row p=li*4+q (li=leaf in group, q=0..3):
            #   q=0,1: bl rows of leaf (v-part rows q of boundary 2g+li+1)
            #   q=2,3: ab rows (u-part rows q-2 of boundary 2g+li)
            Rall = pool.tile([8, 32*S], F32, tag="Rall")
            bl_ap = bleaf[:]
            fsz = 4*S
            for li in range(2):
                for q in range(4):
                    p = li*4 + q
                    if q < 2:
                        # src partition 2g+li+1, free offset (2+q)*S
                        srcoff = (li+1)*fsz + (2+q)*S
                    else:
                        srcoff = li*fsz + (q-2)*S
                    nc.sync.dma_start(
                        _b.AP(Rall[:].tensor, Rall[:].offset + p*(32*S),
                              [[32*S, 1], [S, 32], [1, S]]),
                        _b.AP(bl_ap.tensor, bl_ap.offset + srcoff,
                              [[2*fsz, 32], [1, S]]))
            for g in range(32):
                lm = iopool.tile([8, 128], F32, tag="lm")
                nc.sync.dma_start(lm[:], dins["lmatT"][g])
                rm = iopool.tile([8, S], F32, tag="rm")
                nc.sync.dma_start(rm[:], dins["rmask"][g])
                nc.vector.tensor_tensor(Rall[:, g*S:(g+1)*S], Rall[:, g*S:(g+1)*S], rm[:], MUL)
                ps = pspool.tile([128, S], F32, tag="ps")
                nc.tensor.matmul(ps[:], lm[:], Rall[:, g*S:(g+1)*S])
                ob = iopool.tile([128, S], F32, tag="ob")
                nc.scalar.copy(ob[:], ps[:])
                j = g % 4
                tmpd = iopool.tile([128, 128], F32, tag="tmpd")
                nc.vector.tensor_scalar(tmpd[:], xh[:, j*128:(j+1)*128],
                                        dfl[:, g:g+1], None, MUL)
                nc.vector.tensor_tensor(ob[:, j*128:(j+1)*128],
                                        ob[:, j*128:(j+1)*128], tmpd[:], ADD)
                nc.sync.dma_start(dout[g*128:(g+1)*128, :], ob[:])
    # --- post-pass: this walrus build allows only 1 sync-wait per
    # instruction; split extras onto preceding same-engine NOPs ---
    def _split_waits(maxw=1):
        all_bbs = list(nc.main_func.blocks)
        for bb in all_bbs:
            out = []
            for inst in bb.instructions:
                si = getattr(inst, "sync_info", None)
                ow = list(si.on_wait) if (si is not None and si.on_wait) else []
                if len(ow) > maxw:
                    si.on_wait = ow[-maxw:]
                    try:
                        eng_builder = nc.engines[inst.engine]
                    except Exception:
                        eng_builder = nc.sync
                    for w in ow[:-maxw]:
                        nop = eng_builder.nop()
                        for bb2 in nc.main_func.blocks:
                            li = bb2.instructions
                            if li and li[-1] is nop.ins:
                                li.pop()
                                break
                        nop.ins.sync_info = mybir.SyncInfo(on_wait=[w], on_update=[])
                        out.append(nop.ins)
                out.append(inst)
            bb.instructions[:] = out
    _split_waits()
    return nc, dins, dout


def _device_run(P, timeit=False):
    from concourse.bass_utils import run_bass_kernel_spmd
    if "nc" not in _CACHED:
        _CACHED["nc"] = _build_nc()
    nc, dins, dout = _CACHED["nc"]
    in_maps = [_core_inputs(P, core) for core in range(NCORES)]
    res = run_bass_kernel_spmd(nc, in_maps, list(range(NCORES)))
    slabs = [res.results[c]["xslab"] for c in range(NCORES)]
    return np.concatenate(slabs, axis=1)


def kernel(x, rho, sigma2):
    x = np.asarray(x, dtype=np.float64)
    rho = float(np.asarray(rho)); sigma2 = float(np.asarray(sigma2))
    Bcols = _stage1_bands(x, rho, sigma2)
    bd = _bands_by_diag(Bcols)
    P = _host_pieces(bd)
    _CACHED["P_obj"] = P
    X = _device_run(P).astype(np.float64)
    return X



# revision 2
# speedup vs baseline: 3.6662x; 3.6662x over previous
"""Trainium2 kernel: X = inv(phi + sigma2*A) for the DeepKernelPacketGP module.

Host (f64, O(n) prep): pentadiagonal bands of B via batched 5x5 kernel-packet
window solves; boundary Riccati scans; dressed leaf inverses (Xhat) and
rank-2 propagators (gTLe/gBRe); 252 interface rows of X around the 64-row
leaf boundaries via a banded solve (O(n) per row).
Device (8 cores, column-slab sharding): each core materializes its
X[:, core*512:(core+1)*512] slab as 32 row-block matmuls — each 128-row
block is a rank-4 combination of masked interface rows plus the dressed
diagonal block. Row-block order is rotated per core so the diagonal blocks
always land on program iterations 0..3 (SPMD: one program, per-core data).
"""
import sys
sys.path.insert(0, '/opt/trn_rl_repo')
import numpy as np

N = 4096
NB = 64                    # leaf span size
NLEAF = N // NB            # 64
NCORES = 8
SLAB = N // NCORES         # 512
NGRP = N // 128            # 32 row-groups of 128 rows

# ============================================================================
# Host math (float64)
# ============================================================================

def _stage1_bands(x, rho, sigma2):
    n = x.shape[0]; k = 5; m = 2; n_pow = 2
    c = np.sqrt(3.0) / rho
    W = n - 4
    idx = np.arange(W)[:, None] + np.arange(k)[None, :]
    xw = x[idx]
    t = xw - (xw[:, :1] + xw[:, -1:]) / 2
    pw = t[:, :, None] ** np.arange(n_pow)
    pos = pw * np.exp(c * t)[:, :, None]
    neg = pw * np.exp(-c * t)[:, :, None]
    e_first = np.zeros((W, 1, k)); e_first[:, :, 0] = 1.0
    Amat = np.concatenate([np.swapaxes(pos, 1, 2), np.swapaxes(neg, 1, 2), e_first], axis=1)
    rhs = np.zeros((k,)); rhs[-1] = 1.0
    a = np.linalg.solve(Amat, np.broadcast_to(rhs, (W, k))[..., None])[..., 0]
    d = np.abs(xw[:, :, None] - xw[:, None, :]); s = c * d
    Kw = (1 + s) * np.exp(-s)
    phiv = np.einsum('wij,wj->wi', Kw, a)
    bcol = phiv + sigma2 * a
    Bcols = np.zeros((n, 5))
    Bcols[2:n-2, :] = bcol
    def bnd(xseg, tshift, npos, nneg):
        ss = xseg.shape[0]
        xt = xseg + tshift
        rows = [xt**j * np.exp(c*xt) for j in range(npos)]
        rows += [xt**j * np.exp(-c*xt) for j in range(nneg)]
        e = np.zeros(ss); e[0] = 1.0
        rows.append(e)
        M = np.stack(rows); r = np.zeros(ss); r[-1] = 1.0
        aa = np.linalg.solve(M, r)
        dd = np.abs(xseg[:, None] - xseg[None, :]); s2 = c*dd
        return aa, ((1+s2)*np.exp(-s2)) @ aa
    for i in range(m):
        s_l = i + m + 1
        aa, pp = bnd(x[:s_l], -x[s_l-1], n_pow, s_l - 3)
        for r in range(s_l):
            Bcols[i, r - i + 2] = pp[r] + sigma2*aa[r]
        s_r = k - 1 - i
        aa, pp = bnd(x[n-s_r:], -x[n-s_r], s_r - 3, n_pow)
        col = n - m + i
        for ridx in range(s_r):
            r = n - s_r + ridx
            Bcols[col, r - col + 2] = pp[ridx] + sigma2*aa[ridx]
    return Bcols


def _bands_by_diag(Bcols):
    n = Bcols.shape[0]
    bd = {d: np.zeros(n) for d in range(-2, 3)}
    for j in range(5):
        c0 = max(0, 2 - j); c1 = min(n, n + 2 - j)
        for col in range(c0, c1):
            r = col - 2 + j
            bd[col - r][r] = Bcols[col, j]
    return bd


def _span_matrix(bd, lo, hi):
    s = hi - lo
    M = np.zeros((s, s))
    for d in range(-2, 3):
        r0 = max(0, -d); r1 = min(s, s - d)
        rr = np.arange(r0, r1)
        M[rr, rr + d] = bd[d][lo + rr]
    return M


def _c_coup(bd, b):
    return np.array([[bd[2][b-2], 0.0], [bd[1][b-1], bd[2][b-1]]])


def _e_coup(bd, b):
    return np.array([[bd[-2][b], bd[-1][b]], [0.0, bd[-2][b+1]]])


def _leaf_pieces(bd):
    """Riccati scans + dressed leaf inverses Xhat and propagators gTLe/gBRe."""
    n = N; nl = NLEAF
    GL = np.zeros((nl+1, 2, 2))
    for k in range(1, nl+1):
        lo = (k-1)*NB
        D = _span_matrix(bd, lo, lo+NB)
        if k > 1:
            D[:2, :2] -= _e_coup(bd, lo) @ GL[k-1] @ _c_coup(bd, lo)
        GL[k] = np.linalg.inv(D)[-2:, -2:]
    GR = np.zeros((nl+1, 2, 2))
    for k in range(nl-1, -1, -1):
        lo = k*NB
        D = _span_matrix(bd, lo, lo+NB)
        if k < nl-1:
            b = lo + NB
            D[-2:, -2:] -= _c_coup(bd, b) @ GR[k+1] @ _e_coup(bd, b)
        GR[k] = np.linalg.inv(D)[:2, :2]
    Xhat = np.zeros((nl, NB, NB))
    gTLe = np.zeros((nl, NB, 2))
    gBRe = np.zeros((nl, NB, 2))
    for ell in range(nl):
        lo = ell*NB; hi = lo + NB
        D0 = _span_matrix(bd, lo, hi)
        TLm = np.zeros((NB, NB)); BRm = np.zeros((NB, NB))
        if lo > 0:
            TLm[:2, :2] = _e_coup(bd, lo) @ GL[ell] @ _c_coup(bd, lo)
        if hi < n:
            BRm[-2:, -2:] = _c_coup(bd, hi) @ GR[ell+1] @ _e_coup(bd, hi)
        Xhat[ell] = np.linalg.inv(D0 - TLm - BRm)
        if hi < n:
            gTLe[ell] = -np.linalg.inv(D0 - TLm)[:, -2:] @ _c_coup(bd, hi)
        if lo > 0:
            gBRe[ell] = -np.linalg.inv(D0 - BRm)[:, :2] @ _e_coup(bd, lo)
    return Xhat, gTLe, gBRe


def _interface_rows(bd):
    """Rows X[64k-2 .. 64k+1, :] of X = B^{-1} for k=1..63, via a banded
    solve of B^T Y = E (Y columns are the wanted rows of X)."""
    n = N
    idxs = []
    for k in range(1, NLEAF):
        b = NB * k
        idxs += [b-2, b-1, b, b+1]
    E = np.zeros((n, len(idxs)))
    E[idxs, np.arange(len(idxs))] = 1.0
    # scipy banded form for M = B^T (l=u=2): ab[2+d, j] = M[j+d, j] = B[j, j+d]
    # = bd[d][j]
    try:
        from scipy.linalg import solve_banded
        ab = np.zeros((5, n))
        for d in range(-2, 3):
            ab[2 + d, :] = bd[d]
        Y = solve_banded((2, 2), ab, E)
    except ImportError:
        Bd = np.zeros((n, n))
        for d in range(-2, 3):
            r0 = max(0, -d); r1 = min(n, n - d)
            rr = np.arange(r0, r1)
            Bd[rr, rr + d] = bd[d][rr]
        Y = np.linalg.solve(Bd.T, E)
    R = Y.T  # [252, n]
    rowmap = {r: i for i, r in enumerate(idxs)}
    return R, rowmap


def _core_inputs(R, rowmap, Xhat, gTLe, gBRe, core, in_np, out_np):
    cols = np.arange(core*SLAB, (core+1)*SLAB)
    rall = np.zeros((8, NGRP*SLAB), np.float64)
    lmt = np.zeros((8, NGRP*128), np.float64)
    for t in range(NGRP):
        g = (4*core + t) % NGRP
        for li in range(2):
            ell = 2*g + li
            lo, hi = ell*NB, (ell+1)*NB
            if hi < N:
                msk = (cols >= hi)
                rall[li*4+0, t*SLAB:(t+1)*SLAB] = R[rowmap[hi]][cols] * msk
                rall[li*4+1, t*SLAB:(t+1)*SLAB] = R[rowmap[hi+1]][cols] * msk
            if lo > 0:
                msk = (cols < lo)
                rall[li*4+2, t*SLAB:(t+1)*SLAB] = R[rowmap[lo-2]][cols] * msk
                rall[li*4+3, t*SLAB:(t+1)*SLAB] = R[rowmap[lo-1]][cols] * msk
            blk = np.zeros((NB, 4))
            blk[:, 0:2] = gTLe[ell]     # multiplies the two rows below the leaf
            blk[:, 2:4] = gBRe[ell]     # multiplies the two rows above the leaf
            lmt[li*4:(li+1)*4, t*128 + li*NB: t*128 + (li+1)*NB] = blk.T
    xh = np.zeros((128, 4*128), np.float64)
    for t in range(4):
        g = 4*core + t
        for li in range(2):
            xh[li*NB:(li+1)*NB, t*128 + li*NB: t*128 + (li+1)*NB] = Xhat[2*g+li]
    return {"rall": rall.astype(in_np), "lmt": lmt.astype(in_np),
            "xh": xh.astype(out_np)}


# ============================================================================
# Device kernel
# ============================================================================

IN_DT_NAME = "float32"    # matmul operand dtype
OUT_DT_NAME = "float32"   # output slab dtype

_CACHED = {}


def _dt(mybir, name):
    return getattr(mybir.dt, name)


def _np_dt(name):
    return {"float32": np.float32, "float16": np.float16,
            "bfloat16": None}[name] or __import__("ml_dtypes").bfloat16


def _build_nc():
    import concourse.bass as bass
    import concourse.mybir as mybir
    import concourse.tile as tile
    from concourse.vector_clock import ScopedClock

    def _patched_drain_and_barrier(self, tick_clock, wait_clock):
        nopw = self.nc.gpsimd.nop()
        wait_clock.add_sem_waits(nopw.ins, ScopedClock({None: tick_clock.global_clock}))
        waits = list(nopw.ins.sync_info.on_wait) if nopw.ins.sync_info else []
        if len(waits) > 1:
            nopw.ins.sync_info.on_wait = waits[:1]
            for w in waits[1:]:
                extra = self.nc.gpsimd.nop()
                extra.ins.sync_info = mybir.SyncInfo(on_wait=[w], on_update=[])
        self.nc.sync.drain()
        self.nc.all_engine_barrier()
        assert self.sems is not None
        popped = self.nc._tile_sem_poison_stack.pop()
        assert popped is self._sem_poison
        self.nc.clear_and_free_semaphores(list(self.sems.allocated().values()))
        self.nc.all_engine_barrier()
    tile.TileContext._drain_and_barrier = _patched_drain_and_barrier

    F32 = mybir.dt.float32
    IN_DT = _dt(mybir, IN_DT_NAME)
    OUT_DT = _dt(mybir, OUT_DT_NAME)
    ADD = mybir.AluOpType.add
    S = SLAB

    nc = bass.Bass(target_bir_lowering=False)
    rall_d = nc.dram_tensor("rall", [8, NGRP*S], IN_DT, kind="ExternalInput")
    lmt_d = nc.dram_tensor("lmt", [8, NGRP*128], IN_DT, kind="ExternalInput")
    xh_d = nc.dram_tensor("xh", [128, 4*128], OUT_DT, kind="ExternalInput")
    dout = nc.dram_tensor("xslab", [N, S], OUT_DT, kind="ExternalOutput")

    with tile.TileContext(nc) as tc:
        with tc.tile_pool(name="main", bufs=1) as pool, \
             tc.tile_pool(name="io", bufs=4) as iopool, \
             tc.tile_pool(name="ps", bufs=8, space="PSUM") as pspool:
            rall = pool.tile([8, NGRP*S], IN_DT, tag="rall")
            lmt = pool.tile([8, NGRP*128], IN_DT, tag="lmt")
            xh = pool.tile([128, 4*128], OUT_DT, tag="xh")
            nc.sync.dma_start(lmt[:], lmt_d[:])
            nc.sync.dma_start(xh[:], xh_d[:])
            # split rall DMA so early groups start sooner
            npiece = 8
            per = NGRP // npiece
            for piece in range(npiece):
                nc.sync.dma_start(
                    rall[:, piece*per*S:(piece+1)*per*S],
                    rall_d[:, piece*per*S:(piece+1)*per*S])
            for t in range(NGRP):
                ps = pspool.tile([128, S], F32, tag="ps")
                nc.tensor.matmul(ps[:], lmt[:, t*128:(t+1)*128],
                                 rall[:, t*S:(t+1)*S], start=True, stop=True)
                ob = iopool.tile([128, S], OUT_DT, tag="ob")
                nc.scalar.copy(ob[:], ps[:])
                if t < 4:
                    nc.vector.tensor_tensor(
                        ob[:, t*128:(t+1)*128], ob[:, t*128:(t+1)*128],
                        xh[:, t*128:(t+1)*128], ADD)
                nc.sync.dma_start(dout[t*128:(t+1)*128, :], ob[:])

    # --- post-pass: this walrus build allows only 1 sync-wait per
    # instruction; split extras onto preceding same-engine NOPs ---
    def _split_waits(maxw=1):
        all_bbs = list(nc.main_func.blocks)
        for bb in all_bbs:
            out = []
            for inst in bb.instructions:
                si = getattr(inst, "sync_info", None)
                ow = list(si.on_wait) if (si is not None and si.on_wait) else []
                if len(ow) > maxw:
                    si.on_wait = ow[-maxw:]
                    try:
                        eng_builder = nc.engines[inst.engine]
                    except Exception:
                        eng_builder = nc.sync
                    for w in ow[:-maxw]:
                        nop = eng_builder.nop()
                        for bb2 in nc.main_func.blocks:
                            li = bb2.instructions
                            if li and li[-1] is nop.ins:
                                li.pop()
                                break
                        nop.ins.sync_info = mybir.SyncInfo(on_wait=[w], on_update=[])
                        out.append(nop.ins)
                out.append(inst)
            bb.instructions[:] = out
    _split_waits()
    return nc, dout


def _device_run(in_maps):
    from concourse.bass_utils import run_bass_kernel_spmd
    if "nc" not in _CACHED:
        _CACHED["nc"] = _build_nc()
    nc, dout = _CACHED["nc"]
    res = run_bass_kernel_spmd(nc, in_maps, list(range(NCORES)))
    return [res.results[c]["xslab"] for c in range(NCORES)]


def _assemble(slabs):
    X = np.empty((N, N), dtype=np.float64)
    for core in range(NCORES):
        sl = np.asarray(slabs[core], dtype=np.float64)
        cs = slice(core*SLAB, (core+1)*SLAB)
        for t in range(NGRP):
            g = (4*core + t) % NGRP
            X[g*128:(g+1)*128, cs] = sl[t*128:(t+1)*128, :]
    return X


def kernel(x, rho, sigma2):
    x = np.asarray(x, dtype=np.float64)
    rho = float(np.asarray(rho)); sigma2 = float(np.asarray(sigma2))
    Bcols = _stage1_bands(x, rho, sigma2)
    bd = _bands_by_diag(Bcols)
    Xhat, gTLe, gBRe = _leaf_pieces(bd)
    R, rowmap = _interface_rows(bd)
    in_np = _np_dt(IN_DT_NAME); out_np = _np_dt(OUT_DT_NAME)
    in_maps = [_core_inputs(R, rowmap, Xhat, gTLe, gBRe, c, in_np, out_np)
               for c in range(NCORES)]
    _CACHED["in_maps"] = in_maps
    slabs = _device_run(in_maps)
    return _assemble(slabs)


# revision 4
# speedup vs baseline: 5.7062x; 1.5564x over previous
"""Trainium2 kernel: X = inv(phi + sigma2*A) for the DeepKernelPacketGP module.

Host (f64, O(n) prep): pentadiagonal bands of B via batched 5x5 kernel-packet
window solves; boundary Riccati scans; dressed leaf inverses (Xhat) and
rank-2 propagators (gTLe/gBRe); 252 interface rows of X around the 64-row
leaf boundaries via a banded solve (O(n) per row).
Device (8 cores, column-slab sharding): each core materializes its
X[:, core*512:(core+1)*512] slab as 32 row-block matmuls — each 128-row
block is a rank-4 combination of masked interface rows plus the dressed
diagonal block. Row-block order is rotated per core so the diagonal blocks
always land on program iterations 0..3 (SPMD: one program, per-core data).
"""
import sys
sys.path.insert(0, '/opt/trn_rl_repo')
import numpy as np

N = 4096
NB = 64                    # leaf span size
NLEAF = N // NB            # 64
NCORES = 8
SLAB = N // NCORES         # 512
NGRP = N // 128            # 32 row-groups of 128 rows

# ============================================================================
# Host math (float64)
# ============================================================================

def _stage1_bands(x, rho, sigma2):
    n = x.shape[0]; k = 5; m = 2; n_pow = 2
    c = np.sqrt(3.0) / rho
    W = n - 4
    idx = np.arange(W)[:, None] + np.arange(k)[None, :]
    xw = x[idx]
    t = xw - (xw[:, :1] + xw[:, -1:]) / 2
    pw = t[:, :, None] ** np.arange(n_pow)
    pos = pw * np.exp(c * t)[:, :, None]
    neg = pw * np.exp(-c * t)[:, :, None]
    e_first = np.zeros((W, 1, k)); e_first[:, :, 0] = 1.0
    Amat = np.concatenate([np.swapaxes(pos, 1, 2), np.swapaxes(neg, 1, 2), e_first], axis=1)
    rhs = np.zeros((k,)); rhs[-1] = 1.0
    a = np.linalg.solve(Amat, np.broadcast_to(rhs, (W, k))[..., None])[..., 0]
    d = np.abs(xw[:, :, None] - xw[:, None, :]); s = c * d
    Kw = (1 + s) * np.exp(-s)
    phiv = np.einsum('wij,wj->wi', Kw, a)
    bcol = phiv + sigma2 * a
    Bcols = np.zeros((n, 5))
    Bcols[2:n-2, :] = bcol
    def bnd(xseg, tshift, npos, nneg):
        ss = xseg.shape[0]
        xt = xseg + tshift
        rows = [xt**j * np.exp(c*xt) for j in range(npos)]
        rows += [xt**j * np.exp(-c*xt) for j in range(nneg)]
        e = np.zeros(ss); e[0] = 1.0
        rows.append(e)
        M = np.stack(rows); r = np.zeros(ss); r[-1] = 1.0
        aa = np.linalg.solve(M, r)
        dd = np.abs(xseg[:, None] - xseg[None, :]); s2 = c*dd
        return aa, ((1+s2)*np.exp(-s2)) @ aa
    for i in range(m):
        s_l = i + m + 1
        aa, pp = bnd(x[:s_l], -x[s_l-1], n_pow, s_l - 3)
        for r in range(s_l):
            Bcols[i, r - i + 2] = pp[r] + sigma2*aa[r]
        s_r = k - 1 - i
        aa, pp = bnd(x[n-s_r:], -x[n-s_r], s_r - 3, n_pow)
        col = n - m + i
        for ridx in range(s_r):
            r = n - s_r + ridx
            Bcols[col, r - col + 2] = pp[ridx] + sigma2*aa[ridx]
    return Bcols


def _bands_by_diag(Bcols):
    n = Bcols.shape[0]
    bd = {d: np.zeros(n) for d in range(-2, 3)}
    for j in range(5):
        c0 = max(0, 2 - j); c1 = min(n, n + 2 - j)
        for col in range(c0, c1):
            r = col - 2 + j
            bd[col - r][r] = Bcols[col, j]
    return bd


def _span_matrix(bd, lo, hi):
    s = hi - lo
    M = np.zeros((s, s))
    for d in range(-2, 3):
        r0 = max(0, -d); r1 = min(s, s - d)
        rr = np.arange(r0, r1)
        M[rr, rr + d] = bd[d][lo + rr]
    return M


def _c_coup(bd, b):
    return np.array([[bd[2][b-2], 0.0], [bd[1][b-1], bd[2][b-1]]])


def _e_coup(bd, b):
    return np.array([[bd[-2][b], bd[-1][b]], [0.0, bd[-2][b+1]]])


def _leaf_pieces(bd):
    """Riccati scans + dressed leaf inverses Xhat and propagators gTLe/gBRe."""
    n = N; nl = NLEAF
    GL = np.zeros((nl+1, 2, 2))
    for k in range(1, nl+1):
        lo = (k-1)*NB
        D = _span_matrix(bd, lo, lo+NB)
        if k > 1:
            D[:2, :2] -= _e_coup(bd, lo) @ GL[k-1] @ _c_coup(bd, lo)
        GL[k] = np.linalg.inv(D)[-2:, -2:]
    GR = np.zeros((nl+1, 2, 2))
    for k in range(nl-1, -1, -1):
        lo = k*NB
        D = _span_matrix(bd, lo, lo+NB)
        if k < nl-1:
            b = lo + NB
            D[-2:, -2:] -= _c_coup(bd, b) @ GR[k+1] @ _e_coup(bd, b)
        GR[k] = np.linalg.inv(D)[:2, :2]
    Xhat = np.zeros((nl, NB, NB))
    gTLe = np.zeros((nl, NB, 2))
    gBRe = np.zeros((nl, NB, 2))
    for ell in range(nl):
        lo = ell*NB; hi = lo + NB
        D0 = _span_matrix(bd, lo, hi)
        TLm = np.zeros((NB, NB)); BRm = np.zeros((NB, NB))
        if lo > 0:
            TLm[:2, :2] = _e_coup(bd, lo) @ GL[ell] @ _c_coup(bd, lo)
        if hi < n:
            BRm[-2:, -2:] = _c_coup(bd, hi) @ GR[ell+1] @ _e_coup(bd, hi)
        Xhat[ell] = np.linalg.inv(D0 - TLm - BRm)
        if hi < n:
            gTLe[ell] = -np.linalg.inv(D0 - TLm)[:, -2:] @ _c_coup(bd, hi)
        if lo > 0:
            gBRe[ell] = -np.linalg.inv(D0 - BRm)[:, :2] @ _e_coup(bd, lo)
    return Xhat, gTLe, gBRe


def _interface_rows(bd):
    """Rows X[64k-2 .. 64k+1, :] of X = B^{-1} for k=1..63, via a banded
    solve of B^T Y = E (Y columns are the wanted rows of X)."""
    n = N
    idxs = []
    for k in range(1, NLEAF):
        b = NB * k
        idxs += [b-2, b-1, b, b+1]
    E = np.zeros((n, len(idxs)))
    E[idxs, np.arange(len(idxs))] = 1.0
    # scipy banded form for M = B^T (l=u=2): ab[2+d, j] = M[j+d, j] = B[j, j+d]
    # = bd[d][j]
    try:
        from scipy.linalg import solve_banded
        ab = np.zeros((5, n))
        for d in range(-2, 3):
            ab[2 + d, :] = bd[d]
        Y = solve_banded((2, 2), ab, E)
    except ImportError:
        Bd = np.zeros((n, n))
        for d in range(-2, 3):
            r0 = max(0, -d); r1 = min(n, n - d)
            rr = np.arange(r0, r1)
            Bd[rr, rr + d] = bd[d][rr]
        Y = np.linalg.solve(Bd.T, E)
    R = Y.T  # [252, n]
    rowmap = {r: i for i, r in enumerate(idxs)}
    return R, rowmap


def _ortho_pieces(R, rowmap, Xhat, gTLe, gBRe):
    """Orthonormalize each masked interface-row pair (fold the 2x2 Cholesky
    of the pair Gram into the leaf coefficients). This bounds cancellation in
    the device's rank-4 fp16 matmuls: orthonormal rows mean the product terms
    cannot exceed the row norm of the adjusted coefficients."""
    cols_all = np.arange(N)
    pieces = {}
    for ell in range(NLEAF):
        lo, hi = ell*NB, (ell+1)*NB
        for role in ('bl', 'ab'):
            if role == 'bl':
                if hi >= N:
                    pieces[(ell, role)] = None
                    continue
                mask = cols_all >= hi
                P = np.stack([R[rowmap[hi]], R[rowmap[hi+1]]]) * mask
                g = gTLe[ell]
            else:
                if lo == 0:
                    pieces[(ell, role)] = None
                    continue
                mask = cols_all < lo
                P = np.stack([R[rowmap[lo-2]], R[rowmap[lo-1]]]) * mask
                g = gBRe[ell]
            L = np.linalg.cholesky(P @ P.T)
            pieces[(ell, role)] = (np.linalg.solve(L, P), g @ L)
    growmax = max(np.linalg.norm(r[1], axis=1).max()
                  for r in pieces.values() if r is not None)
    Xbound = max(np.abs(Xhat).max(), np.sqrt(2.0)*growmax)
    s_out = 15000.0 / Xbound
    s_r = 30000.0
    return pieces, s_r, s_out


def _core_inputs(pieces, Xhat, s_r, s_out, core, in_np, out_np):
    cols = slice(core*SLAB, (core+1)*SLAB)
    s_l = s_out / s_r
    rall = np.zeros((8, NGRP*SLAB), np.float64)
    lmt = np.zeros((8, NGRP*128), np.float64)
    for t in range(NGRP):
        g = (4*core + t) % NGRP
        for li in range(2):
            ell = 2*g + li
            pb = pieces[(ell, 'bl')]
            pa = pieces[(ell, 'ab')]
            if pb is not None:
                rall[li*4+0:li*4+2, t*SLAB:(t+1)*SLAB] = pb[0][:, cols] * s_r
                lmt[li*4:li*4+2, t*128 + li*NB: t*128 + (li+1)*NB] = pb[1].T * s_l
            if pa is not None:
                rall[li*4+2:li*4+4, t*SLAB:(t+1)*SLAB] = pa[0][:, cols] * s_r
                lmt[li*4+2:li*4+4, t*128 + li*NB: t*128 + (li+1)*NB] = pa[1].T * s_l
    xh = np.zeros((128, 4*128), np.float64)
    for t in range(4):
        g = 4*core + t
        for li in range(2):
            xh[li*NB:(li+1)*NB, t*128 + li*NB: t*128 + (li+1)*NB] = Xhat[2*g+li] * s_out
    return {"rall": rall.astype(in_np), "lmt": lmt.astype(in_np),
            "xh": xh.astype(out_np)}


# ============================================================================
# Device kernel
# ============================================================================

IN_DT_NAME = "float16"    # matmul operand dtype
OUT_DT_NAME = "float16"   # output slab dtype

_CACHED = {}


def _dt(mybir, name):
    return getattr(mybir.dt, name)


def _np_dt(name):
    return {"float32": np.float32, "float16": np.float16,
            "bfloat16": None}[name] or __import__("ml_dtypes").bfloat16


def _build_nc():
    import concourse.bass as bass
    import concourse.mybir as mybir
    import concourse.tile as tile
    from concourse.vector_clock import ScopedClock

    def _patched_drain_and_barrier(self, tick_clock, wait_clock):
        nopw = self.nc.gpsimd.nop()
        wait_clock.add_sem_waits(nopw.ins, ScopedClock({None: tick_clock.global_clock}))
        waits = list(nopw.ins.sync_info.on_wait) if nopw.ins.sync_info else []
        if len(waits) > 1:
            nopw.ins.sync_info.on_wait = waits[:1]
            for w in waits[1:]:
                extra = self.nc.gpsimd.nop()
                extra.ins.sync_info = mybir.SyncInfo(on_wait=[w], on_update=[])
        self.nc.sync.drain()
        self.nc.all_engine_barrier()
        assert self.sems is not None
        popped = self.nc._tile_sem_poison_stack.pop()
        assert popped is self._sem_poison
        self.nc.clear_and_free_semaphores(list(self.sems.allocated().values()))
        self.nc.all_engine_barrier()
    tile.TileContext._drain_and_barrier = _patched_drain_and_barrier

    F32 = mybir.dt.float32
    IN_DT = _dt(mybir, IN_DT_NAME)
    OUT_DT = _dt(mybir, OUT_DT_NAME)
    ADD = mybir.AluOpType.add
    S = SLAB

    nc = bass.Bass(target_bir_lowering=False)
    rall_d = nc.dram_tensor("rall", [8, NGRP*S], IN_DT, kind="ExternalInput")
    lmt_d = nc.dram_tensor("lmt", [8, NGRP*128], IN_DT, kind="ExternalInput")
    xh_d = nc.dram_tensor("xh", [128, 4*128], OUT_DT, kind="ExternalInput")
    dout = nc.dram_tensor("xslab", [N, S], OUT_DT, kind="ExternalOutput")

    with tile.TileContext(nc) as tc:
        with tc.tile_pool(name="main", bufs=1) as pool, \
             tc.tile_pool(name="io", bufs=4) as iopool, \
             tc.tile_pool(name="ps", bufs=8, space="PSUM") as pspool:
            rall = pool.tile([8, NGRP*S], IN_DT, tag="rall")
            lmt = pool.tile([8, NGRP*128], IN_DT, tag="lmt")
            xh = pool.tile([128, 4*128], OUT_DT, tag="xh")
            nc.sync.dma_start(lmt[:], lmt_d[:])
            nc.sync.dma_start(xh[:], xh_d[:])
            # split rall DMA so early groups start sooner
            npiece = 8
            per = NGRP // npiece
            for piece in range(npiece):
                nc.sync.dma_start(
                    rall[:, piece*per*S:(piece+1)*per*S],
                    rall_d[:, piece*per*S:(piece+1)*per*S])
            for t in range(NGRP):
                ps = pspool.tile([128, S], F32, tag="ps")
                nc.tensor.matmul(ps[:], lmt[:, t*128:(t+1)*128],
                                 rall[:, t*S:(t+1)*S], start=True, stop=True)
                ob = iopool.tile([128, S], OUT_DT, tag="ob")
                if t % 2 == 0:
                    nc.scalar.copy(ob[:], ps[:])
                else:
                    nc.vector.tensor_scalar_mul(ob[:], ps[:], 1.0)
                if t < 4:
                    nc.vector.tensor_tensor(
                        ob[:, t*128:(t+1)*128], ob[:, t*128:(t+1)*128],
                        xh[:, t*128:(t+1)*128], ADD)
                nc.sync.dma_start(dout[t*128:(t+1)*128, :], ob[:])

    # --- post-pass: this walrus build allows only 1 sync-wait per
    # instruction; split extras onto preceding same-engine NOPs ---
    def _split_waits(maxw=1):
        all_bbs = list(nc.main_func.blocks)
        for bb in all_bbs:
            out = []
            for inst in bb.instructions:
                si = getattr(inst, "sync_info", None)
                ow = list(si.on_wait) if (si is not None and si.on_wait) else []
                if len(ow) > maxw:
                    si.on_wait = ow[-maxw:]
                    try:
                        eng_builder = nc.engines[inst.engine]
                    except Exception:
                        eng_builder = nc.sync
                    for w in ow[:-maxw]:
                        nop = eng_builder.nop()
                        for bb2 in nc.main_func.blocks:
                            li = bb2.instructions
                            if li and li[-1] is nop.ins:
                                li.pop()
                                break
                        nop.ins.sync_info = mybir.SyncInfo(on_wait=[w], on_update=[])
                        out.append(nop.ins)
                out.append(inst)
            bb.instructions[:] = out
    _split_waits()
    return nc, dout


def _device_run(in_maps):
    from concourse.bass_utils import run_bass_kernel_spmd
    if "nc" not in _CACHED:
        _CACHED["nc"] = _build_nc()
    nc, dout = _CACHED["nc"]
    res = run_bass_kernel_spmd(nc, in_maps, list(range(NCORES)))
    return [res.results[c]["xslab"] for c in range(NCORES)]


def _assemble(slabs, s_out):
    X = np.empty((N, N), dtype=np.float64)
    for core in range(NCORES):
        sl = np.asarray(slabs[core], dtype=np.float64)
        cs = slice(core*SLAB, (core+1)*SLAB)
        for t in range(NGRP):
            g = (4*core + t) % NGRP
            X[g*128:(g+1)*128, cs] = sl[t*128:(t+1)*128, :]
    X *= 1.0 / s_out
    return X


def kernel(x, rho, sigma2):
    x = np.asarray(x, dtype=np.float64)
    rho = float(np.asarray(rho)); sigma2 = float(np.asarray(sigma2))
    Bcols = _stage1_bands(x, rho, sigma2)
    bd = _bands_by_diag(Bcols)
    Xhat, gTLe, gBRe = _leaf_pieces(bd)
    R, rowmap = _interface_rows(bd)
    pieces, s_r, s_out = _ortho_pieces(R, rowmap, Xhat, gTLe, gBRe)
    in_np = _np_dt(IN_DT_NAME); out_np = _np_dt(OUT_DT_NAME)
    in_maps = [_core_inputs(pieces, Xhat, s_r, s_out, c, in_np, out_np)
               for c in range(NCORES)]
    _CACHED["in_maps"] = in_maps
    slabs = _device_run(in_maps)
    return _assemble(slabs, s_out)


# revision 6
# speedup vs baseline: 7.4389x; 1.3037x over previous
"""Trainium2 kernel: X = inv(phi + sigma2*A) for the DeepKernelPacketGP module.

Host (f64, O(n) prep): pentadiagonal bands of B via batched 5x5 kernel-packet
window solves; boundary Riccati scans; dressed leaf inverses (Xhat) and
rank-2 propagators (gTLe/gBRe); 252 interface rows of X around the 64-row
leaf boundaries via a banded solve (O(n) per row).
Device (8 cores, column-slab sharding): each core materializes its
X[:, core*512:(core+1)*512] slab as 32 row-block matmuls — each 128-row
block is a rank-4 combination of masked interface rows plus the dressed
diagonal block. Row-block order is rotated per core so the diagonal blocks
always land on program iterations 0..3 (SPMD: one program, per-core data).
"""
import sys
sys.path.insert(0, '/opt/trn_rl_repo')
import numpy as np

N = 4096
NB = 64                    # leaf span size
NLEAF = N // NB            # 64
NCORES = 8
SLAB = N // NCORES         # 512
NGRP = N // 128            # 32 row-groups of 128 rows

# ============================================================================
# Host math (float64)
# ============================================================================

def _stage1_bands(x, rho, sigma2):
    n = x.shape[0]; k = 5; m = 2; n_pow = 2
    c = np.sqrt(3.0) / rho
    W = n - 4
    idx = np.arange(W)[:, None] + np.arange(k)[None, :]
    xw = x[idx]
    t = xw - (xw[:, :1] + xw[:, -1:]) / 2
    pw = t[:, :, None] ** np.arange(n_pow)
    pos = pw * np.exp(c * t)[:, :, None]
    neg = pw * np.exp(-c * t)[:, :, None]
    e_first = np.zeros((W, 1, k)); e_first[:, :, 0] = 1.0
    Amat = np.concatenate([np.swapaxes(pos, 1, 2), np.swapaxes(neg, 1, 2), e_first], axis=1)
    rhs = np.zeros((k,)); rhs[-1] = 1.0
    a = np.linalg.solve(Amat, np.broadcast_to(rhs, (W, k))[..., None])[..., 0]
    d = np.abs(xw[:, :, None] - xw[:, None, :]); s = c * d
    Kw = (1 + s) * np.exp(-s)
    phiv = np.einsum('wij,wj->wi', Kw, a)
    bcol = phiv + sigma2 * a
    Bcols = np.zeros((n, 5))
    Bcols[2:n-2, :] = bcol
    def bnd(xseg, tshift, npos, nneg):
        ss = xseg.shape[0]
        xt = xseg + tshift
        rows = [xt**j * np.exp(c*xt) for j in range(npos)]
        rows += [xt**j * np.exp(-c*xt) for j in range(nneg)]
        e = np.zeros(ss); e[0] = 1.0
        rows.append(e)
        M = np.stack(rows); r = np.zeros(ss); r[-1] = 1.0
        aa = np.linalg.solve(M, r)
        dd = np.abs(xseg[:, None] - xseg[None, :]); s2 = c*dd
        return aa, ((1+s2)*np.exp(-s2)) @ aa
    for i in range(m):
        s_l = i + m + 1
        aa, pp = bnd(x[:s_l], -x[s_l-1], n_pow, s_l - 3)
        for r in range(s_l):
            Bcols[i, r - i + 2] = pp[r] + sigma2*aa[r]
        s_r = k - 1 - i
        aa, pp = bnd(x[n-s_r:], -x[n-s_r], s_r - 3, n_pow)
        col = n - m + i
        for ridx in range(s_r):
            r = n - s_r + ridx
            Bcols[col, r - col + 2] = pp[ridx] + sigma2*aa[ridx]
    return Bcols


def _bands_by_diag(Bcols):
    n = Bcols.shape[0]
    bd = {d: np.zeros(n) for d in range(-2, 3)}
    for j in range(5):
        c0 = max(0, 2 - j); c1 = min(n, n + 2 - j)
        for col in range(c0, c1):
            r = col - 2 + j
            bd[col - r][r] = Bcols[col, j]
    return bd


def _span_matrix(bd, lo, hi):
    s = hi - lo
    M = np.zeros((s, s))
    for d in range(-2, 3):
        r0 = max(0, -d); r1 = min(s, s - d)
        rr = np.arange(r0, r1)
        M[rr, rr + d] = bd[d][lo + rr]
    return M


def _c_coup(bd, b):
    return np.array([[bd[2][b-2], 0.0], [bd[1][b-1], bd[2][b-1]]])


def _e_coup(bd, b):
    return np.array([[bd[-2][b], bd[-1][b]], [0.0, bd[-2][b+1]]])


def _leaf_pieces(bd):
    """Riccati scans + dressed leaf inverses Xhat and propagators gTLe/gBRe."""
    n = N; nl = NLEAF
    GL = np.zeros((nl+1, 2, 2))
    for k in range(1, nl+1):
        lo = (k-1)*NB
        D = _span_matrix(bd, lo, lo+NB)
        if k > 1:
            D[:2, :2] -= _e_coup(bd, lo) @ GL[k-1] @ _c_coup(bd, lo)
        GL[k] = np.linalg.inv(D)[-2:, -2:]
    GR = np.zeros((nl+1, 2, 2))
    for k in range(nl-1, -1, -1):
        lo = k*NB
        D = _span_matrix(bd, lo, lo+NB)
        if k < nl-1:
            b = lo + NB
            D[-2:, -2:] -= _c_coup(bd, b) @ GR[k+1] @ _e_coup(bd, b)
        GR[k] = np.linalg.inv(D)[:2, :2]
    Xhat = np.zeros((nl, NB, NB))
    gTLe = np.zeros((nl, NB, 2))
    gBRe = np.zeros((nl, NB, 2))
    for ell in range(nl):
        lo = ell*NB; hi = lo + NB
        D0 = _span_matrix(bd, lo, hi)
        TLm = np.zeros((NB, NB)); BRm = np.zeros((NB, NB))
        if lo > 0:
            TLm[:2, :2] = _e_coup(bd, lo) @ GL[ell] @ _c_coup(bd, lo)
        if hi < n:
            BRm[-2:, -2:] = _c_coup(bd, hi) @ GR[ell+1] @ _e_coup(bd, hi)
        Xhat[ell] = np.linalg.inv(D0 - TLm - BRm)
        if hi < n:
            gTLe[ell] = -np.linalg.inv(D0 - TLm)[:, -2:] @ _c_coup(bd, hi)
        if lo > 0:
            gBRe[ell] = -np.linalg.inv(D0 - BRm)[:, :2] @ _e_coup(bd, lo)
    return Xhat, gTLe, gBRe


def _interface_rows(bd):
    """Rows X[64k-2 .. 64k+1, :] of X = B^{-1} for k=1..63, via a banded
    solve of B^T Y = E (Y columns are the wanted rows of X)."""
    n = N
    idxs = []
    for k in range(1, NLEAF):
        b = NB * k
        idxs += [b-2, b-1, b, b+1]
    E = np.zeros((n, len(idxs)))
    E[idxs, np.arange(len(idxs))] = 1.0
    # scipy banded form for M = B^T (l=u=2): ab[2+d, j] = M[j+d, j] = B[j, j+d]
    # = bd[d][j]
    try:
        from scipy.linalg import solve_banded
        ab = np.zeros((5, n))
        for d in range(-2, 3):
            ab[2 + d, :] = bd[d]
        Y = solve_banded((2, 2), ab, E)
    except ImportError:
        Bd = np.zeros((n, n))
        for d in range(-2, 3):
            r0 = max(0, -d); r1 = min(n, n - d)
            rr = np.arange(r0, r1)
            Bd[rr, rr + d] = bd[d][rr]
        Y = np.linalg.solve(Bd.T, E)
    R = Y.T  # [252, n]
    rowmap = {r: i for i, r in enumerate(idxs)}
    return R, rowmap


def _ortho_pieces(R, rowmap, Xhat, gTLe, gBRe):
    """Orthonormalize each masked interface-row pair (fold the 2x2 Cholesky
    of the pair Gram into the leaf coefficients). This bounds cancellation in
    the device's rank-4 fp16 matmuls: orthonormal rows mean the product terms
    cannot exceed the row norm of the adjusted coefficients."""
    cols_all = np.arange(N)
    pieces = {}
    for ell in range(NLEAF):
        lo, hi = ell*NB, (ell+1)*NB
        for role in ('bl', 'ab'):
            if role == 'bl':
                if hi >= N:
                    pieces[(ell, role)] = None
                    continue
                mask = cols_all >= hi
                P = np.stack([R[rowmap[hi]], R[rowmap[hi+1]]]) * mask
                g = gTLe[ell]
            else:
                if lo == 0:
                    pieces[(ell, role)] = None
                    continue
                mask = cols_all < lo
                P = np.stack([R[rowmap[lo-2]], R[rowmap[lo-1]]]) * mask
                g = gBRe[ell]
            L = np.linalg.cholesky(P @ P.T)
            pieces[(ell, role)] = (np.linalg.solve(L, P), g @ L)
    growmax = max(np.linalg.norm(r[1], axis=1).max()
                  for r in pieces.values() if r is not None)
    Xbound = max(np.abs(Xhat).max(), np.sqrt(2.0)*growmax)
    s_out = 15000.0 / Xbound
    s_r = 30000.0
    return pieces, s_r, s_out


def _core_inputs(pieces, Xhat, s_r, s_out, core, in_np, out_np):
    cols = slice(core*SLAB, (core+1)*SLAB)
    s_l = s_out / s_r
    rall = np.zeros((8, NGRP*SLAB), np.float64)
    lmt = np.zeros((8, NGRP*128), np.float64)
    for t in range(NGRP):
        g = (4*core + t) % NGRP
        for li in range(2):
            ell = 2*g + li
            pb = pieces[(ell, 'bl')]
            pa = pieces[(ell, 'ab')]
            if pb is not None:
                rall[li*4+0:li*4+2, t*SLAB:(t+1)*SLAB] = pb[0][:, cols] * s_r
                lmt[li*4:li*4+2, t*128 + li*NB: t*128 + (li+1)*NB] = pb[1].T * s_l
            if pa is not None:
                rall[li*4+2:li*4+4, t*SLAB:(t+1)*SLAB] = pa[0][:, cols] * s_r
                lmt[li*4+2:li*4+4, t*128 + li*NB: t*128 + (li+1)*NB] = pa[1].T * s_l
    xh = np.zeros((128, 4*128), np.float64)
    for t in range(4):
        g = 4*core + t
        for li in range(2):
            xh[li*NB:(li+1)*NB, t*128 + li*NB: t*128 + (li+1)*NB] = Xhat[2*g+li] * s_out
    return {"rall": rall.astype(in_np), "lmt": lmt.astype(in_np),
            "xh": xh.astype(out_np)}


# ============================================================================
# Device kernel
# ============================================================================

IN_DT_NAME = "float16"    # matmul operand dtype
OUT_DT_NAME = "float16"   # output slab dtype

_CACHED = {}


def _dt(mybir, name):
    return getattr(mybir.dt, name)


def _np_dt(name):
    return {"float32": np.float32, "float16": np.float16,
            "bfloat16": None}[name] or __import__("ml_dtypes").bfloat16


def _build_nc():
    import concourse.bass as bass
    import concourse.mybir as mybir
    import concourse.tile as tile
    from concourse.vector_clock import ScopedClock

    def _patched_drain_and_barrier(self, tick_clock, wait_clock):
        nopw = self.nc.gpsimd.nop()
        wait_clock.add_sem_waits(nopw.ins, ScopedClock({None: tick_clock.global_clock}))
        waits = list(nopw.ins.sync_info.on_wait) if nopw.ins.sync_info else []
        if len(waits) > 1:
            nopw.ins.sync_info.on_wait = waits[:1]
            for w in waits[1:]:
                extra = self.nc.gpsimd.nop()
                extra.ins.sync_info = mybir.SyncInfo(on_wait=[w], on_update=[])
        self.nc.sync.drain()
        self.nc.all_engine_barrier()
        assert self.sems is not None
        popped = self.nc._tile_sem_poison_stack.pop()
        assert popped is self._sem_poison
        self.nc.clear_and_free_semaphores(list(self.sems.allocated().values()))
        self.nc.all_engine_barrier()
    tile.TileContext._drain_and_barrier = _patched_drain_and_barrier

    F32 = mybir.dt.float32
    IN_DT = _dt(mybir, IN_DT_NAME)
    OUT_DT = _dt(mybir, OUT_DT_NAME)
    ADD = mybir.AluOpType.add
    S = SLAB

    nc = bass.Bass(target_bir_lowering=False)
    rall_d = nc.dram_tensor("rall", [8, NGRP*S], IN_DT, kind="ExternalInput")
    lmt_d = nc.dram_tensor("lmt", [8, NGRP*128], IN_DT, kind="ExternalInput")
    xh_d = nc.dram_tensor("xh", [128, 4*128], OUT_DT, kind="ExternalInput")
    dout = nc.dram_tensor("xslab", [N, S], OUT_DT, kind="ExternalOutput")

    with tile.TileContext(nc) as tc:
        with tc.tile_pool(name="main", bufs=1) as pool, \
             tc.tile_pool(name="io", bufs=4) as iopool, \
             tc.tile_pool(name="ps", bufs=4, space="PSUM") as pspool:
            import concourse.bass as _b
            rall = pool.tile([8, NGRP*S], IN_DT, tag="rall")
            lmt = pool.tile([8, NGRP*128], IN_DT, tag="lmt")
            xh = pool.tile([128, 4*128], OUT_DT, tag="xh")
            # input DMAs on the scalar hw-DGE ring; it is otherwise idle at
            # kernel start while sync handles output issue later
            nc.scalar.dma_start(rall[:], rall_d[:])
            nc.sync.dma_start(lmt[:], lmt_d[:])
            nc.sync.dma_start(xh[:], xh_d[:])
            ps = None
            obq = None
            for t in range(NGRP):
                if t % 2 == 0:
                    ps = pspool.tile([128, 2*S], F32, tag="ps")
                if t % 4 == 0:
                    obq = iopool.tile([128, 4*S], OUT_DT, tag="obq")
                half = (t % 2) * S
                nc.tensor.matmul(ps[:, half:half+S], lmt[:, t*128:(t+1)*128],
                                 rall[:, t*S:(t+1)*S], start=True, stop=True)
                if t % 2 == 1:
                    # evict the completed psum pair into the quad staging tile
                    pairoff = ((t % 4) // 2) * 2 * S
                    if (t // 2) % 2 == 0:
                        nc.scalar.copy(obq[:, pairoff:pairoff+2*S], ps[:])
                    else:
                        nc.vector.tensor_scalar_mul(
                            obq[:, pairoff:pairoff+2*S], ps[:], 1.0)
                    if t < 4:
                        for td in (t-1, t):
                            off = td*S + td*128
                            nc.vector.tensor_tensor(
                                obq[:, off:off+128], obq[:, off:off+128],
                                xh[:, td*128:(td+1)*128], ADD)
                if t % 4 == 3:
                    # dst iterates (p, chunk, col) to match the SBUF tile's
                    # natural (partition, free) order; chunk c is row block
                    # q*512 + c*128 + p of the slab
                    q = t // 4
                    dst = _b.AP(dout[0:1, 0:1].tensor, q*4*128*S,
                                [[S, 128], [128*S, 4], [1, S]])
                    nc.sync.dma_start(dst, obq[:])

    # --- post-pass: this walrus build allows only 1 sync-wait per
    # instruction; split extras onto preceding same-engine NOPs ---
    def _split_waits(maxw=1):
        all_bbs = list(nc.main_func.blocks)
        for bb in all_bbs:
            out = []
            for inst in bb.instructions:
                si = getattr(inst, "sync_info", None)
                ow = list(si.on_wait) if (si is not None and si.on_wait) else []
                if len(ow) > maxw:
                    si.on_wait = ow[-maxw:]
                    try:
                        eng_builder = nc.engines[inst.engine]
                    except Exception:
                        eng_builder = nc.sync
                    for w in ow[:-maxw]:
                        nop = eng_builder.nop()
                        for bb2 in nc.main_func.blocks:
                            li = bb2.instructions
                            if li and li[-1] is nop.ins:
                                li.pop()
                                break
                        nop.ins.sync_info = mybir.SyncInfo(on_wait=[w], on_update=[])
                        out.append(nop.ins)
                out.append(inst)
            bb.instructions[:] = out
    _split_waits()
    return nc, dout


def _device_run(in_maps):
    from concourse.bass_utils import run_bass_kernel_spmd
    if "nc" not in _CACHED:
        _CACHED["nc"] = _build_nc()
    nc, dout = _CACHED["nc"]
    res = run_bass_kernel_spmd(nc, in_maps, list(range(NCORES)))
    return [res.results[c]["xslab"] for c in range(NCORES)]


def _assemble(slabs, s_out):
    X = np.empty((N, N), dtype=np.float64)
    for core in range(NCORES):
        sl = np.asarray(slabs[core], dtype=np.float64)
        cs = slice(core*SLAB, (core+1)*SLAB)
        for t in range(NGRP):
            g = (4*core + t) % NGRP
            X[g*128:(g+1)*128, cs] = sl[t*128:(t+1)*128, :]
    X *= 1.0 / s_out
    return X


def kernel(x, rho, sigma2):
    x = np.asarray(x, dtype=np.float64)
    rho = float(np.asarray(rho)); sigma2 = float(np.asarray(sigma2))
    Bcols = _stage1_bands(x, rho, sigma2)
    bd = _bands_by_diag(Bcols)
    Xhat, gTLe, gBRe = _leaf_pieces(bd)
    R, rowmap = _interface_rows(bd)
    pieces, s_r, s_out = _ortho_pieces(R, rowmap, Xhat, gTLe, gBRe)
    in_np = _np_dt(IN_DT_NAME); out_np = _np_dt(OUT_DT_NAME)
    in_maps = [_core_inputs(pieces, Xhat, s_r, s_out, c, in_np, out_np)
               for c in range(NCORES)]
    _CACHED["in_maps"] = in_maps
    slabs = _device_run(in_maps)
    return _assemble(slabs, s_out)


# revision 9
# speedup vs baseline: 7.7286x; 1.0390x over previous
"""Trainium2 kernel: X = inv(phi + sigma2*A) for the DeepKernelPacketGP module.

Host (f64, O(n) prep): pentadiagonal bands of B via batched 5x5 kernel-packet
window solves; boundary Riccati scans; dressed leaf inverses (Xhat) and
rank-2 propagators (gTLe/gBRe); 252 interface rows of X around the 64-row
leaf boundaries via a banded solve (O(n) per row).
Device (8 cores, column-slab sharding): each core materializes its
X[:, core*512:(core+1)*512] slab as 32 row-block matmuls — each 128-row
block is a rank-4 combination of masked interface rows plus the dressed
diagonal block. Row-block order is rotated per core so the diagonal blocks
always land on program iterations 0..3 (SPMD: one program, per-core data).
"""
import sys
sys.path.insert(0, '/opt/trn_rl_repo')
import numpy as np

N = 4096
NB = 64                    # leaf span size
NLEAF = N // NB            # 64
NCORES = 8
SLAB = N // NCORES         # 512
NGRP = N // 128            # 32 row-groups of 128 rows

# ============================================================================
# Host math (float64)
# ============================================================================

def _stage1_bands(x, rho, sigma2):
    n = x.shape[0]; k = 5; m = 2; n_pow = 2
    c = np.sqrt(3.0) / rho
    W = n - 4
    idx = np.arange(W)[:, None] + np.arange(k)[None, :]
    xw = x[idx]
    t = xw - (xw[:, :1] + xw[:, -1:]) / 2
    pw = t[:, :, None] ** np.arange(n_pow)
    pos = pw * np.exp(c * t)[:, :, None]
    neg = pw * np.exp(-c * t)[:, :, None]
    e_first = np.zeros((W, 1, k)); e_first[:, :, 0] = 1.0
    Amat = np.concatenate([np.swapaxes(pos, 1, 2), np.swapaxes(neg, 1, 2), e_first], axis=1)
    rhs = np.zeros((k,)); rhs[-1] = 1.0
    a = np.linalg.solve(Amat, np.broadcast_to(rhs, (W, k))[..., None])[..., 0]
    d = np.abs(xw[:, :, None] - xw[:, None, :]); s = c * d
    Kw = (1 + s) * np.exp(-s)
    phiv = np.einsum('wij,wj->wi', Kw, a)
    bcol = phiv + sigma2 * a
    Bcols = np.zeros((n, 5))
    Bcols[2:n-2, :] = bcol
    def bnd(xseg, tshift, npos, nneg):
        ss = xseg.shape[0]
        xt = xseg + tshift
        rows = [xt**j * np.exp(c*xt) for j in range(npos)]
        rows += [xt**j * np.exp(-c*xt) for j in range(nneg)]
        e = np.zeros(ss); e[0] = 1.0
        rows.append(e)
        M = np.stack(rows); r = np.zeros(ss); r[-1] = 1.0
        aa = np.linalg.solve(M, r)
        dd = np.abs(xseg[:, None] - xseg[None, :]); s2 = c*dd
        return aa, ((1+s2)*np.exp(-s2)) @ aa
    for i in range(m):
        s_l = i + m + 1
        aa, pp = bnd(x[:s_l], -x[s_l-1], n_pow, s_l - 3)
        for r in range(s_l):
            Bcols[i, r - i + 2] = pp[r] + sigma2*aa[r]
        s_r = k - 1 - i
        aa, pp = bnd(x[n-s_r:], -x[n-s_r], s_r - 3, n_pow)
        col = n - m + i
        for ridx in range(s_r):
            r = n - s_r + ridx
            Bcols[col, r - col + 2] = pp[ridx] + sigma2*aa[ridx]
    return Bcols


def _bands_by_diag(Bcols):
    n = Bcols.shape[0]
    bd = {d: np.zeros(n) for d in range(-2, 3)}
    for j in range(5):
        c0 = max(0, 2 - j); c1 = min(n, n + 2 - j)
        for col in range(c0, c1):
            r = col - 2 + j
            bd[col - r][r] = Bcols[col, j]
    return bd


def _span_matrix(bd, lo, hi):
    s = hi - lo
    M = np.zeros((s, s))
    for d in range(-2, 3):
        r0 = max(0, -d); r1 = min(s, s - d)
        rr = np.arange(r0, r1)
        M[rr, rr + d] = bd[d][lo + rr]
    return M


def _c_coup(bd, b):
    return np.array([[bd[2][b-2], 0.0], [bd[1][b-1], bd[2][b-1]]])


def _e_coup(bd, b):
    return np.array([[bd[-2][b], bd[-1][b]], [0.0, bd[-2][b+1]]])


def _leaf_pieces(bd):
    """Riccati scans + dressed leaf inverses Xhat and propagators gTLe/gBRe."""
    n = N; nl = NLEAF
    GL = np.zeros((nl+1, 2, 2))
    for k in range(1, nl+1):
        lo = (k-1)*NB
        D = _span_matrix(bd, lo, lo+NB)
        if k > 1:
            D[:2, :2] -= _e_coup(bd, lo) @ GL[k-1] @ _c_coup(bd, lo)
        GL[k] = np.linalg.inv(D)[-2:, -2:]
    GR = np.zeros((nl+1, 2, 2))
    for k in range(nl-1, -1, -1):
        lo = k*NB
        D = _span_matrix(bd, lo, lo+NB)
        if k < nl-1:
            b = lo + NB
            D[-2:, -2:] -= _c_coup(bd, b) @ GR[k+1] @ _e_coup(bd, b)
        GR[k] = np.linalg.inv(D)[:2, :2]
    Xhat = np.zeros((nl, NB, NB))
    gTLe = np.zeros((nl, NB, 2))
    gBRe = np.zeros((nl, NB, 2))
    for ell in range(nl):
        lo = ell*NB; hi = lo + NB
        D0 = _span_matrix(bd, lo, hi)
        TLm = np.zeros((NB, NB)); BRm = np.zeros((NB, NB))
        if lo > 0:
            TLm[:2, :2] = _e_coup(bd, lo) @ GL[ell] @ _c_coup(bd, lo)
        if hi < n:
            BRm[-2:, -2:] = _c_coup(bd, hi) @ GR[ell+1] @ _e_coup(bd, hi)
        Xhat[ell] = np.linalg.inv(D0 - TLm - BRm)
        if hi < n:
            gTLe[ell] = -np.linalg.inv(D0 - TLm)[:, -2:] @ _c_coup(bd, hi)
        if lo > 0:
            gBRe[ell] = -np.linalg.inv(D0 - BRm)[:, :2] @ _e_coup(bd, lo)
    return Xhat, gTLe, gBRe


def _interface_rows(bd):
    """Rows X[64k-2 .. 64k+1, :] of X = B^{-1} for k=1..63, via a banded
    solve of B^T Y = E (Y columns are the wanted rows of X)."""
    n = N
    idxs = []
    for k in range(1, NLEAF):
        b = NB * k
        idxs += [b-2, b-1, b, b+1]
    E = np.zeros((n, len(idxs)))
    E[idxs, np.arange(len(idxs))] = 1.0
    # scipy banded form for M = B^T (l=u=2): ab[2+d, j] = M[j+d, j] = B[j, j+d]
    # = bd[d][j]
    try:
        from scipy.linalg import solve_banded
        ab = np.zeros((5, n))
        for d in range(-2, 3):
            ab[2 + d, :] = bd[d]
        Y = solve_banded((2, 2), ab, E)
    except ImportError:
        Bd = np.zeros((n, n))
        for d in range(-2, 3):
            r0 = max(0, -d); r1 = min(n, n - d)
            rr = np.arange(r0, r1)
            Bd[rr, rr + d] = bd[d][rr]
        Y = np.linalg.solve(Bd.T, E)
    R = Y.T  # [252, n]
    rowmap = {r: i for i, r in enumerate(idxs)}
    return R, rowmap


def _ortho_pieces(R, rowmap, Xhat, gTLe, gBRe):
    """Orthonormalize each masked interface-row pair (fold the 2x2 Cholesky
    of the pair Gram into the leaf coefficients). This bounds cancellation in
    the device's rank-4 fp16 matmuls: orthonormal rows mean the product terms
    cannot exceed the row norm of the adjusted coefficients."""
    cols_all = np.arange(N)
    pieces = {}
    for ell in range(NLEAF):
        lo, hi = ell*NB, (ell+1)*NB
        for role in ('bl', 'ab'):
            if role == 'bl':
                if hi >= N:
                    pieces[(ell, role)] = None
                    continue
                mask = cols_all >= hi
                P = np.stack([R[rowmap[hi]], R[rowmap[hi+1]]]) * mask
                g = gTLe[ell]
            else:
                if lo == 0:
                    pieces[(ell, role)] = None
                    continue
                mask = cols_all < lo
                P = np.stack([R[rowmap[lo-2]], R[rowmap[lo-1]]]) * mask
                g = gBRe[ell]
            L = np.linalg.cholesky(P @ P.T)
            pieces[(ell, role)] = (np.linalg.solve(L, P), g @ L)
    growmax = max(np.linalg.norm(r[1], axis=1).max()
                  for r in pieces.values() if r is not None)
    Xbound = max(np.abs(Xhat).max(), np.sqrt(2.0)*growmax)
    s_out = 15000.0 / Xbound
    s_r = 30000.0
    return pieces, s_r, s_out


def _core_inputs(pieces, Xhat, s_r, s_out, core, in_np, out_np):
    """Inputs packed for K=32 quadrant matmuls: group t's 8 interface rows sit
    at partitions 8*(t%16), its coefficients at the same partitions in a
    distinct free slice; the matmul contracts over a full 32-partition PE
    quadrant with zero coefficient rows masking the three co-resident groups.
    All 128 SBUF partitions carry data, so the input DMA writes ~2KB per
    partition instead of 32KB on 8 partitions."""
    cols = slice(core*SLAB, (core+1)*SLAB)
    s_l = s_out / s_r
    rall = np.zeros((128, 2*SLAB), np.float64)
    lmt = np.zeros((128, 16*128), np.float64)
    for t in range(NGRP):
        g = (4*core + t) % NGRP
        P = 8*(t % 16)
        F = (t//16)*SLAB
        L = ((t//16)*8 + (t % 8))*128
        for li in range(2):
            ell = 2*g + li
            pb = pieces[(ell, 'bl')]
            pa = pieces[(ell, 'ab')]
            if pb is not None:
                rall[P+li*4:P+li*4+2, F:F+SLAB] = pb[0][:, cols] * s_r
                lmt[P+li*4:P+li*4+2, L + li*NB: L + (li+1)*NB] = pb[1].T * s_l
            if pa is not None:
                rall[P+li*4+2:P+li*4+4, F:F+SLAB] = pa[0][:, cols] * s_r
                lmt[P+li*4+2:P+li*4+4, L + li*NB: L + (li+1)*NB] = pa[1].T * s_l
    xh = np.zeros((128, 4*128), np.float64)
    for t in range(4):
        g = 4*core + t
        for li in range(2):
            xh[li*NB:(li+1)*NB, t*128 + li*NB: t*128 + (li+1)*NB] = Xhat[2*g+li] * s_out
    return {"rall": rall.astype(in_np), "lmt": lmt.astype(in_np),
            "xh": xh.astype(out_np)}


# ============================================================================
# Device kernel
# ============================================================================

IN_DT_NAME = "float16"    # matmul operand dtype
OUT_DT_NAME = "float16"   # output slab dtype

_CACHED = {}


def _dt(mybir, name):
    return getattr(mybir.dt, name)


def _np_dt(name):
    return {"float32": np.float32, "float16": np.float16,
            "bfloat16": None}[name] or __import__("ml_dtypes").bfloat16


def _build_nc():
    import concourse.bass as bass
    import concourse.mybir as mybir
    import concourse.tile as tile
    from concourse.vector_clock import ScopedClock

    def _patched_drain_and_barrier(self, tick_clock, wait_clock):
        nopw = self.nc.gpsimd.nop()
        wait_clock.add_sem_waits(nopw.ins, ScopedClock({None: tick_clock.global_clock}))
        waits = list(nopw.ins.sync_info.on_wait) if nopw.ins.sync_info else []
        if len(waits) > 1:
            nopw.ins.sync_info.on_wait = waits[:1]
            for w in waits[1:]:
                extra = self.nc.gpsimd.nop()
                extra.ins.sync_info = mybir.SyncInfo(on_wait=[w], on_update=[])
        self.nc.sync.drain()
        self.nc.all_engine_barrier()
        assert self.sems is not None
        popped = self.nc._tile_sem_poison_stack.pop()
        assert popped is self._sem_poison
        self.nc.clear_and_free_semaphores(list(self.sems.allocated().values()))
        self.nc.all_engine_barrier()
    tile.TileContext._drain_and_barrier = _patched_drain_and_barrier

    F32 = mybir.dt.float32
    IN_DT = _dt(mybir, IN_DT_NAME)
    OUT_DT = _dt(mybir, OUT_DT_NAME)
    ADD = mybir.AluOpType.add
    S = SLAB

    nc = bass.Bass(target_bir_lowering=False)
    rall_d = nc.dram_tensor("rall", [128, 2*S], IN_DT, kind="ExternalInput")
    lmt_d = nc.dram_tensor("lmt", [128, 16*128], IN_DT, kind="ExternalInput")
    xh_d = nc.dram_tensor("xh", [128, 4*128], OUT_DT, kind="ExternalInput")
    dout = nc.dram_tensor("xslab", [N, S], OUT_DT, kind="ExternalOutput")

    with tile.TileContext(nc) as tc:
        with tc.tile_pool(name="main", bufs=1) as pool, \
             tc.tile_pool(name="io", bufs=4) as iopool, \
             tc.tile_pool(name="ps", bufs=4, space="PSUM") as pspool:
            import concourse.bass as _b
            rall = pool.tile([128, 2*S], IN_DT, tag="rall")
            lmt = pool.tile([128, 16*128], IN_DT, tag="lmt")
            xh = pool.tile([128, 4*128], OUT_DT, tag="xh")
            # input DMAs on the scalar hw-DGE ring; it is otherwise idle at
            # kernel start while sync handles output issue later
            nc.scalar.dma_start(rall[:], rall_d[:])
            nc.sync.dma_start(lmt[:], lmt_d[:])
            nc.sync.dma_start(xh[:], xh_d[:])
            ps = None
            obq = None
            for t in range(NGRP):
                if t % 2 == 0:
                    ps = pspool.tile([128, 2*S], F32, tag="ps")
                if t % 4 == 0:
                    obq = iopool.tile([128, 4*S], OUT_DT, tag="obq")
                half = (t % 2) * S
                B = 64*((t % 16)//8)
                F = (t//16)*S
                L = ((t//16)*8 + (t % 8))*128
                nc.tensor.matmul(ps[:, half:half+S], lmt[B:B+64, L:L+128],
                                 rall[B:B+64, F:F+S], start=True, stop=True)
                if t % 2 == 1:
                    # evict the completed psum pair into the quad staging tile
                    pairoff = ((t % 4) // 2) * 2 * S
                    if (t // 2) % 2 == 0:
                        nc.scalar.copy(obq[:, pairoff:pairoff+2*S], ps[:])
                    else:
                        nc.vector.tensor_scalar_mul(
                            obq[:, pairoff:pairoff+2*S], ps[:], 1.0)
                    if t < 4:
                        for td in (t-1, t):
                            off = td*S + td*128
                            nc.vector.tensor_tensor(
                                obq[:, off:off+128], obq[:, off:off+128],
                                xh[:, td*128:(td+1)*128], ADD)
                if t % 4 == 3:
                    # dst iterates (p, chunk, col) to match the SBUF tile's
                    # natural (partition, free) order; chunk c is row block
                    # q*512 + c*128 + p of the slab
                    q = t // 4
                    dst = _b.AP(dout[0:1, 0:1].tensor, q*4*128*S,
                                [[S, 128], [128*S, 4], [1, S]])
                    nc.sync.dma_start(dst, obq[:])

    # --- post-pass: this walrus build allows only 1 sync-wait per
    # instruction; split extras onto preceding same-engine NOPs ---
    def _split_waits(maxw=1):
        all_bbs = list(nc.main_func.blocks)
        for bb in all_bbs:
            out = []
            for inst in bb.instructions:
                si = getattr(inst, "sync_info", None)
                ow = list(si.on_wait) if (si is not None and si.on_wait) else []
                if len(ow) > maxw:
                    si.on_wait = ow[-maxw:]
                    try:
                        eng_builder = nc.engines[inst.engine]
                    except Exception:
                        eng_builder = nc.sync
                    for w in ow[:-maxw]:
                        nop = eng_builder.nop()
                        for bb2 in nc.main_func.blocks:
                            li = bb2.instructions
                            if li and li[-1] is nop.ins:
                                li.pop()
                                break
                        nop.ins.sync_info = mybir.SyncInfo(on_wait=[w], on_update=[])
                        out.append(nop.ins)
                out.append(inst)
            bb.instructions[:] = out
    _split_waits()
    return nc, dout


def _device_run(in_maps):
    from concourse.bass_utils import run_bass_kernel_spmd
    if "nc" not in _CACHED:
        _CACHED["nc"] = _build_nc()
    nc, dout = _CACHED["nc"]
    res = run_bass_kernel_spmd(nc, in_maps, list(range(NCORES)))
    return [res.results[c]["xslab"] for c in range(NCORES)]


def _assemble(slabs, s_out):
    X = np.empty((N, N), dtype=np.float64)
    for core in range(NCORES):
        sl = np.asarray(slabs[core], dtype=np.float64)
        cs = slice(core*SLAB, (core+1)*SLAB)
        for t in range(NGRP):
            g = (4*core + t) % NGRP
            X[g*128:(g+1)*128, cs] = sl[t*128:(t+1)*128, :]
    X *= 1.0 / s_out
    return X


def kernel(x, rho, sigma2):
    x = np.asarray(x, dtype=np.float64)
    rho = float(np.asarray(rho)); sigma2 = float(np.asarray(sigma2))
    Bcols = _stage1_bands(x, rho, sigma2)
    bd = _bands_by_diag(Bcols)
    Xhat, gTLe, gBRe = _leaf_pieces(bd)
    R, rowmap = _interface_rows(bd)
    pieces, s_r, s_out = _ortho_pieces(R, rowmap, Xhat, gTLe, gBRe)
    in_np = _np_dt(IN_DT_NAME); out_np = _np_dt(OUT_DT_NAME)
    in_maps = [_core_inputs(pieces, Xhat, s_r, s_out, c, in_np, out_np)
               for c in range(NCORES)]
    _CACHED["in_maps"] = in_maps
    slabs = _device_run(in_maps)
    return _assemble(slabs, s_out)
